# revision 5
# baseline (speedup 1.0000x reference)
"""kNN graph construction (N=4 sets, M=4096 points, D=128, k=16) on 8 trn2 cores.

Sharding: core c handles point set c//2, row half c%2 (2048 query rows x 4096
candidates).  Each core's input set is rotated so its rows come first; the SPMD
program is identical across cores and host code un-rotates returned indices.

Per-core device program:
  - load x_set [4096,128] f32, PE-transpose to xT [128,4096] (d on partitions)
  - split xT (and -x2/2) into fp32r (e8m11) hi + lo parts on chip
  - PSUM s = rank-1(bias_hi) + rank-1(bias_lo) + hi.hi + hi.lo + lo.hi
    => s[i,j] = x_i . x_j - |x_j|^2/2 at ~fp32 precision, a monotone
    transform of -dist(i,j); fp32r matmuls run at 1 cycle/row (vs 4 for f32)
  - ACT evicts PSUM -> SBUF
  - DVE top-16 per row: max per 512-chunk (8x) -> 64 candidates; max /
    match_replace / max on candidates -> rank 1-8 and 9-16 values; two
    full-row max_index calls recover indices (first-match = lowest index,
    matching jax.lax.top_k tie-breaking)
  - DMA idx [128,16] u32 per row-block to DRAM
"""

import os
import sys

import numpy as np

for _p in (os.environ.get("TRN_RL_REPO"), "/opt/trn_rl_repo"):
    if _p and _p not in sys.path and os.path.isdir(_p):
        sys.path.insert(0, _p)

N_SETS = 4
M = 4096
D = 128
K = 16
N_CORES = 8
ROWS_PER_CORE = M // 2  # 2048
ROW_TILES = ROWS_PER_CORE // 128  # 16
CHUNK = 512
N_CHUNKS = M // CHUNK  # 8
NEG_INF = -1.0e30

_compiled = None


def build_program():
    import concourse.bacc as bacc
    import concourse.mybir as mybir
    import concourse.tile as tile

    f32 = mybir.dt.float32
    f32r = mybir.dt.float32r
    u32 = mybir.dt.uint32

    nc = bacc.Bacc(
        "TRN2",
        target_bir_lowering=False,
        debug=False,
        enable_asserts=False,
    )

    xin = nc.dram_tensor("xin", [M, D], f32, kind="ExternalInput").ap()
    ident = nc.dram_tensor("ident", [128, 128], f32, kind="ExternalInput").ap()
    negx2_in = nc.dram_tensor("negx2", [1, M], f32, kind="ExternalInput").ap()
    idx_out = nc.dram_tensor(
        "idx_out", [ROWS_PER_CORE, K], u32, kind="ExternalOutput"
    ).ap()

    with tile.TileContext(nc) as tc:
        with tc.tile_pool(name="const", bufs=1) as constp:
            ident_sb = constp.tile([128, 128], f32)
            nc.sync.dma_start(ident_sb[:], ident[:, :])
            ones_f32 = constp.tile([1, 128], f32)
            nc.vector.memset(ones_f32[:], 1.0)
            ones_row = constp.tile([1, 128], f32r)
            nc.scalar.copy(ones_row[:], ones_f32[:])  # f32r-rounded producer
            negx2 = constp.tile([1, M], f32)
            nc.sync.dma_start(negx2[:], negx2_in[:, :])
            xT = constp.tile([128, M], f32)

            # transpose x [4096,128] -> xT [128,4096] via PE, 128x128 blocks
            with tc.tile_pool(name="stage", bufs=4) as stagep, tc.tile_pool(
                name="tpsum", bufs=2, space="PSUM"
            ) as tpsum:
                for b in range(M // 128):
                    xa = stagep.tile([128, 128], f32)
                    nc.sync.dma_start(xa[:], xin[b * 128 : (b + 1) * 128, :])
                    pt = tpsum.tile([128, 128], f32)
                    nc.tensor.transpose(pt[:], xa[:], ident_sb[:])
                    nc.scalar.copy(xT[:, b * 128 : (b + 1) * 128], pt[:])

            # split into fp32r (e8m11) hi/lo: x = hi + lo at ~fp32 precision
            xTh = constp.tile([128, M], f32r)
            xTl = constp.tile([128, M], f32r)
            scratch = constp.tile([128, M], f32)
            nc.scalar.copy(xTh[:], xT[:])  # rounds f32 -> f32r
            nc.vector.tensor_sub(scratch[:], xT[:], xTh[:].bitcast(f32))
            nc.scalar.copy(xTl[:], scratch[:])
            nx2h = constp.tile([1, M], f32r)
            nx2l = constp.tile([1, M], f32r)
            scr1 = constp.tile([1, M], f32)
            nc.scalar.copy(nx2h[:], negx2[:])
            nc.vector.tensor_sub(scr1[:], negx2[:], nx2h[:].bitcast(f32))
            nc.scalar.copy(nx2l[:], scr1[:])

            with tc.tile_pool(name="mm", bufs=2, space="PSUM") as mmp, tc.tile_pool(
                name="sbuf_s", bufs=3
            ) as sp, tc.tile_pool(name="small", bufs=3) as smallp:
                for t in range(ROW_TILES):
                    s_sb = sp.tile([128, M], f32, tag="s")
                    rh = xTh[:, t * 128 : (t + 1) * 128]
                    rl = xTl[:, t * 128 : (t + 1) * 128]
                    for h in range(2):
                        ps = mmp.tile([128, 2048], f32, tag="ps")
                        for q in range(4):
                            cs = h * 2048 + q * 512
                            pslice = ps[:, q * 512 : (q + 1) * 512]
                            nc.tensor.matmul(
                                pslice,
                                lhsT=ones_row[:],
                                rhs=nx2h[0:1, cs : cs + 512],
                                start=True,
                                stop=False,
                            )
                            nc.tensor.matmul(
                                pslice,
                                lhsT=ones_row[:],
                                rhs=nx2l[0:1, cs : cs + 512],
                                start=False,
                                stop=False,
                            )
                            ch = xTh[:, cs : cs + 512]
                            cl = xTl[:, cs : cs + 512]
                            nc.tensor.matmul(
                                pslice, lhsT=rh, rhs=ch, start=False, stop=False
                            )
                            nc.tensor.matmul(
                                pslice, lhsT=rh, rhs=cl, start=False, stop=False
                            )
                            nc.tensor.matmul(
                                pslice, lhsT=rl, rhs=ch, start=False, stop=True
                            )
                        nc.scalar.copy(s_sb[:, h * 2048 : (h + 1) * 2048], ps[:])

                    cand = smallp.tile([128, 8 * N_CHUNKS], f32, tag="cand")
                    for c in range(N_CHUNKS):
                        nc.vector.max(
                            cand[:, c * 8 : (c + 1) * 8],
                            s_sb[:, c * CHUNK : (c + 1) * CHUNK],
                        )
                    f8a = smallp.tile([128, 8], f32, tag="f8a")
                    nc.vector.max(f8a[:], cand[:])
                    cand_mr = smallp.tile([128, 8 * N_CHUNKS], f32, tag="cmr")
                    nc.vector.match_replace(
                        out=cand_mr[:],
                        in_to_replace=f8a[:],
                        in_values=cand[:],
                        imm_value=NEG_INF,
                    )
                    f8b = smallp.tile([128, 8], f32, tag="f8b")
                    nc.vector.max(f8b[:], cand_mr[:])

                    idx16 = smallp.tile([128, K], u32, tag="idx")
                    nc.vector.max_index(idx16[:, 0:8], f8a[:], s_sb[:])
                    nc.vector.max_index(idx16[:, 8:16], f8b[:], s_sb[:])
                    nc.sync.dma_start(
                        idx_out[t * 128 : (t + 1) * 128, :], idx16[:]
                    )

    nc.compile()
    return nc


def get_program():
    global _compiled
    if _compiled is None:
        _compiled = build_program()
    return _compiled


def make_in_maps(x):
    """x: [N_SETS, M, D] float32 -> list of 8 per-core input dicts."""
    x = np.asarray(x, dtype=np.float32)
    ident = np.eye(128, dtype=np.float32)
    in_maps = []
    for c in range(N_CORES):
        s, half = divmod(c, 2)
        xs = x[s]
        if half:
            xs = np.concatenate([xs[ROWS_PER_CORE:], xs[:ROWS_PER_CORE]], axis=0)
        xs = np.ascontiguousarray(xs)
        x2 = np.einsum("md,md->m", xs, xs, dtype=np.float32).astype(np.float32)
        negx2 = (-0.5 * x2).reshape(1, M).astype(np.float32)
        in_maps.append({"xin": xs, "ident": ident, "negx2": negx2})
    return in_maps


def assemble(per_core_idx):
    """per_core_idx: list of 8 [2048,16] u32 arrays -> (src, dst) int32."""
    src = np.empty((N_SETS, M, K), dtype=np.int64)
    for c in range(N_CORES):
        s, half = divmod(c, 2)
        idx = per_core_idx[c].astype(np.int64)
        local = (idx + half * ROWS_PER_CORE) % M
        src[s, half * ROWS_PER_CORE : (half + 1) * ROWS_PER_CORE, :] = local + s * M
    src = src.reshape(-1).astype(np.int32)
    dst = np.repeat(np.arange(N_SETS * M, dtype=np.int32), K)
    return src, dst


def run_spmd(x, trace=False, **kwargs):
    from concourse import bass_utils

    nc = get_program()
    in_maps = make_in_maps(x)
    res = bass_utils.run_bass_kernel_spmd(
        nc, in_maps, core_ids=list(range(N_CORES)), trace=trace, **kwargs
    )
    per_core = [res.results[c]["idx_out"] for c in range(N_CORES)]
    return assemble(per_core), res


def kernel(x, k):
    k = int(np.asarray(k))
    assert k == K, f"kernel hardcoded for k={K}, got {k}"
    x = np.asarray(x, dtype=np.float32)
    assert x.shape == (N_SETS, M, D), f"unexpected shape {x.shape}"
    (src, dst), _ = run_spmd(x)
    return src, dst


# revision 20
# speedup vs baseline: 1.4068x; 1.4068x over previous
"""kNN graph construction (N=4 sets, M=4096 points, D=128, k=16) on 8 trn2 cores.

Sharding: core c handles point set c//2, row half c%2 (2048 query rows x 4096
candidates).  Each core's input set is rotated so its rows come first; the SPMD
program is identical across cores and host code un-rotates returned indices.

Per-core device program:
  - load x_set [4096,128] f32, PE-transpose to xT [128,4096] (d on partitions)
  - split xT into fp32r (e8m11) hi + lo parts on chip
  - per 512-col chunk: one bf16 K=3 matmul (ones3^T @ bias3, where bias3 is
    the host-side 3-term bf16 split of -x2/2, exact to 2^-24) opens the PSUM
    group, then hi.hi + hi.lo + lo.hi fp32r matmuls accumulate on top
    => s[i,j] = x_i . x_j - |x_j|^2/2 at ~fp32 precision, a monotone
    transform of -dist(i,j)
  - ACT evicts PSUM -> SBUF
  - DVE top-16 per row: max per 512-chunk (8x) -> 64 candidates; max /
    match_replace / max on candidates -> rank 1-8 and 9-16 values; two
    full-row max_index calls recover indices (first-match = lowest index,
    matching jax.lax.top_k tie-breaking)
  - DMA idx [128,16] u32 per row-block to DRAM
"""

import os
import sys

import ml_dtypes
import numpy as np

for _p in (os.environ.get("TRN_RL_REPO"), "/opt/trn_rl_repo"):
    if _p and _p not in sys.path and os.path.isdir(_p):
        sys.path.insert(0, _p)

N_SETS = 4
M = 4096
D = 128
K = 16
N_CORES = 8
ROWS_PER_CORE = M // 2  # 2048
ROW_TILES = ROWS_PER_CORE // 128  # 16
CHUNK = 512
N_CHUNKS = M // CHUNK  # 8
NEG_INF = -1.0e30

_compiled = None


def build_program():
    import concourse.bacc as bacc
    import concourse.mybir as mybir
    import concourse.tile as tile

    f32 = mybir.dt.float32
    f32r = mybir.dt.float32r
    bf16 = mybir.dt.bfloat16
    u32 = mybir.dt.uint32

    nc = bacc.Bacc(
        "TRN2",
        target_bir_lowering=False,
        debug=False,
        enable_asserts=False,
    )

    xin = nc.dram_tensor("xin", [M, D], f32, kind="ExternalInput").ap()
    ident = nc.dram_tensor("ident", [128, 128], f32, kind="ExternalInput").ap()
    negx2_in = nc.dram_tensor("negx2b3", [3, M], mybir.dt.bfloat16, kind="ExternalInput").ap()
    idx_out = nc.dram_tensor(
        "idx_out", [ROWS_PER_CORE, K], u32, kind="ExternalOutput"
    ).ap()

    with tile.TileContext(nc) as tc:
        with tc.tile_pool(name="const", bufs=1) as constp:
            ident_sb = constp.tile([128, 128], f32)
            nc.sync.dma_start(ident_sb[:], ident[:, :])
            negx2sb = constp.tile([3, M], bf16)
            nc.sync.dma_start(negx2sb[:], negx2_in[:, :])
            ones3 = constp.tile([3, 128], bf16)
            nc.vector.memset(ones3[:], 1.0)
            xT = constp.tile([128, M], f32)

            # transpose x [4096,128] -> xT [128,4096] via PE, 128x128 blocks
            with tc.tile_pool(name="stage", bufs=4) as stagep, tc.tile_pool(
                name="tpsum", bufs=2, space="PSUM"
            ) as tpsum:
                for b in range(M // 128):
                    xa = stagep.tile([128, 128], f32)
                    nc.sync.dma_start(xa[:], xin[b * 128 : (b + 1) * 128, :])
                    pt = tpsum.tile([128, 128], f32)
                    nc.tensor.transpose(pt[:], xa[:], ident_sb[:])
                    nc.scalar.copy(xT[:, b * 128 : (b + 1) * 128], pt[:])

            # split into fp32r (e8m11) hi/lo: x = hi + lo at ~fp32 precision
            xTh = constp.tile([128, M], f32r)
            xTl = constp.tile([128, M], f32r)
            scratch = constp.tile([128, M], f32)
            nc.scalar.copy(xTh[:], xT[:])  # rounds f32 -> f32r
            nc.vector.tensor_sub(scratch[:], xT[:], xTh[:].bitcast(f32))
            nc.scalar.copy(xTl[:], scratch[:])

            with tc.tile_pool(name="mm", bufs=2, space="PSUM") as mmp, tc.tile_pool(
                name="sbuf_s", bufs=3
            ) as sp, tc.tile_pool(name="small", bufs=3) as smallp:
                for t in range(ROW_TILES):
                    s_sb = sp.tile([128, M], f32, tag="s")
                    rh = xTh[:, t * 128 : (t + 1) * 128]
                    rl = xTl[:, t * 128 : (t + 1) * 128]
                    for h in range(2):
                        ps = mmp.tile([128, 2048], f32, tag="ps")
                        for q in range(4):
                            cs = h * 2048 + q * 512
                            pslice = ps[:, q * 512 : (q + 1) * 512]
                            nc.tensor.matmul(
                                pslice,
                                lhsT=ones3[:],
                                rhs=negx2sb[:, cs : cs + 512],
                                start=True,
                                stop=False,
                            )
                            ch = xTh[:, cs : cs + 512]
                            cl = xTl[:, cs : cs + 512]
                            nc.tensor.matmul(
                                pslice, lhsT=rh, rhs=ch, start=False, stop=False
                            )
                            nc.tensor.matmul(
                                pslice, lhsT=rh, rhs=cl, start=False, stop=False
                            )
                            nc.tensor.matmul(
                                pslice, lhsT=rl, rhs=ch, start=False, stop=True
                            )
                        nc.scalar.copy(s_sb[:, h * 2048 : (h + 1) * 2048], ps[:])

                    cand = smallp.tile([128, 8 * N_CHUNKS], f32, tag="cand")
                    for c in range(N_CHUNKS):
                        nc.vector.max(
                            cand[:, c * 8 : (c + 1) * 8],
                            s_sb[:, c * CHUNK : (c + 1) * CHUNK],
                        )
                    f8a = smallp.tile([128, 8], f32, tag="f8a")
                    nc.vector.max(f8a[:], cand[:])
                    cand_mr = smallp.tile([128, 8 * N_CHUNKS], f32, tag="cmr")
                    nc.vector.match_replace(
                        out=cand_mr[:],
                        in_to_replace=f8a[:],
                        in_values=cand[:],
                        imm_value=NEG_INF,
                    )
                    f8b = smallp.tile([128, 8], f32, tag="f8b")
                    nc.vector.max(f8b[:], cand_mr[:])

                    idx16 = smallp.tile([128, K], u32, tag="idx")
                    nc.vector.max_index(idx16[:, 0:8], f8a[:], s_sb[:])
                    nc.vector.max_index(idx16[:, 8:16], f8b[:], s_sb[:])
                    nc.sync.dma_start(
                        idx_out[t * 128 : (t + 1) * 128, :], idx16[:]
                    )

    nc.compile()
    return nc


def get_program():
    global _compiled
    if _compiled is None:
        _compiled = build_program()
    return _compiled


def make_in_maps(x):
    """x: [N_SETS, M, D] float32 -> list of 8 per-core input dicts."""
    x = np.asarray(x, dtype=np.float32)
    ident = np.eye(128, dtype=np.float32)
    in_maps = []
    for c in range(N_CORES):
        s, half = divmod(c, 2)
        xs = x[s]
        if half:
            xs = np.concatenate([xs[ROWS_PER_CORE:], xs[:ROWS_PER_CORE]], axis=0)
        xs = np.ascontiguousarray(xs)
        x2 = np.einsum("md,md->m", xs, xs, dtype=np.float32).astype(np.float32)
        b = (-0.5 * x2).astype(np.float32)
        b3 = np.zeros((3, M), dtype=ml_dtypes.bfloat16)
        rem = b
        for i in range(3):
            b3[i] = rem.astype(ml_dtypes.bfloat16)
            rem = rem - b3[i].astype(np.float32)
        in_maps.append({"xin": xs, "ident": ident, "negx2b3": b3})
    return in_maps


def assemble(per_core_idx):
    """per_core_idx: list of 8 [2048,16] u32 arrays -> (src, dst) int32."""
    src = np.empty((N_SETS, M, K), dtype=np.int64)
    for c in range(N_CORES):
        s, half = divmod(c, 2)
        idx = per_core_idx[c].astype(np.int64)
        local = (idx + half * ROWS_PER_CORE) % M
        src[s, half * ROWS_PER_CORE : (half + 1) * ROWS_PER_CORE, :] = local + s * M
    src = src.reshape(-1).astype(np.int32)
    dst = np.repeat(np.arange(N_SETS * M, dtype=np.int32), K)
    return src, dst


def run_spmd(x, trace=False, **kwargs):
    from concourse import bass_utils

    nc = get_program()
    in_maps = make_in_maps(x)
    res = bass_utils.run_bass_kernel_spmd(
        nc, in_maps, core_ids=list(range(N_CORES)), trace=trace, **kwargs
    )
    per_core = [res.results[c]["idx_out"] for c in range(N_CORES)]
    return assemble(per_core), res


def kernel(x, k):
    k = int(np.asarray(k))
    assert k == K, f"kernel hardcoded for k={K}, got {k}"
    x = np.asarray(x, dtype=np.float32)
    assert x.shape == (N_SETS, M, D), f"unexpected shape {x.shape}"
    (src, dst), _ = run_spmd(x)
    return src, dst


# revision 21
# speedup vs baseline: 1.4831x; 1.0542x over previous
"""kNN graph construction (N=4 sets, M=4096 points, D=128, k=16) on 8 trn2 cores.

Sharding: core c handles point set c//2, row half c%2 (2048 query rows x 4096
candidates).  Each core's input set is rotated so its rows come first; the SPMD
program is identical across cores and host code un-rotates returned indices.

Per-core device program:
  - load xT [128,4096] f32 (transposed on host: d on partitions)
  - split xT into fp32r (e8m11) hi + lo parts on chip
  - per 512-col chunk: one bf16 K=3 matmul (ones3^T @ bias3, where bias3 is
    the host-side 3-term bf16 split of -x2/2, exact to 2^-24) opens the PSUM
    group, then hi.hi + hi.lo + lo.hi fp32r matmuls accumulate on top
    => s[i,j] = x_i . x_j - |x_j|^2/2 at ~fp32 precision, a monotone
    transform of -dist(i,j)
  - ACT evicts PSUM -> SBUF
  - DVE top-16 per row: max per 512-chunk (8x) -> 64 candidates; max /
    match_replace / max on candidates -> rank 1-8 and 9-16 values; two
    full-row max_index calls recover indices (first-match = lowest index,
    matching jax.lax.top_k tie-breaking)
  - DMA idx [128,16] u32 per row-block to DRAM
"""

import os
import sys

import ml_dtypes
import numpy as np

for _p in (os.environ.get("TRN_RL_REPO"), "/opt/trn_rl_repo"):
    if _p and _p not in sys.path and os.path.isdir(_p):
        sys.path.insert(0, _p)

N_SETS = 4
M = 4096
D = 128
K = 16
N_CORES = 8
ROWS_PER_CORE = M // 2  # 2048
ROW_TILES = ROWS_PER_CORE // 128  # 16
CHUNK = 512
N_CHUNKS = M // CHUNK  # 8
NEG_INF = -1.0e30

_compiled = None


def build_program():
    import concourse.bacc as bacc
    import concourse.mybir as mybir
    import concourse.tile as tile

    f32 = mybir.dt.float32
    f32r = mybir.dt.float32r
    bf16 = mybir.dt.bfloat16
    u32 = mybir.dt.uint32

    nc = bacc.Bacc(
        "TRN2",
        target_bir_lowering=False,
        debug=False,
        enable_asserts=False,
    )

    xt_in = nc.dram_tensor("xt", [128, M], f32, kind="ExternalInput").ap()
    negx2_in = nc.dram_tensor("negx2b3", [3, M], mybir.dt.bfloat16, kind="ExternalInput").ap()
    idx_out = nc.dram_tensor(
        "idx_out", [ROWS_PER_CORE, K], u32, kind="ExternalOutput"
    ).ap()

    with tile.TileContext(nc) as tc:
        with tc.tile_pool(name="const", bufs=1) as constp:
            negx2sb = constp.tile([3, M], bf16)
            nc.sync.dma_start(negx2sb[:], negx2_in[:, :])
            ones3 = constp.tile([3, 128], bf16)
            nc.vector.memset(ones3[:], 1.0)
            xT = constp.tile([128, M], f32)
            nc.sync.dma_start(xT[:], xt_in[:, :])

            # split into fp32r (e8m11) hi/lo: x = hi + lo at ~fp32 precision
            xTh = constp.tile([128, M], f32r)
            xTl = constp.tile([128, M], f32r)
            scratch = constp.tile([128, M], f32)
            nc.scalar.copy(xTh[:], xT[:])  # rounds f32 -> f32r
            nc.vector.tensor_sub(scratch[:], xT[:], xTh[:].bitcast(f32))
            nc.scalar.copy(xTl[:], scratch[:])

            with tc.tile_pool(name="mm", bufs=2, space="PSUM") as mmp, tc.tile_pool(
                name="sbuf_s", bufs=4
            ) as sp, tc.tile_pool(name="small", bufs=3) as smallp:
                for t in range(ROW_TILES):
                    s_sb = sp.tile([128, M], f32, tag="s")
                    rh = xTh[:, t * 128 : (t + 1) * 128]
                    rl = xTl[:, t * 128 : (t + 1) * 128]
                    for h in range(2):
                        ps = mmp.tile([128, 2048], f32, tag="ps")
                        for q in range(4):
                            cs = h * 2048 + q * 512
                            pslice = ps[:, q * 512 : (q + 1) * 512]
                            nc.tensor.matmul(
                                pslice,
                                lhsT=ones3[:],
                                rhs=negx2sb[:, cs : cs + 512],
                                start=True,
                                stop=False,
                            )
                            ch = xTh[:, cs : cs + 512]
                            cl = xTl[:, cs : cs + 512]
                            nc.tensor.matmul(
                                pslice, lhsT=rh, rhs=ch, start=False, stop=False
                            )
                            nc.tensor.matmul(
                                pslice, lhsT=rh, rhs=cl, start=False, stop=False
                            )
                            nc.tensor.matmul(
                                pslice, lhsT=rl, rhs=ch, start=False, stop=True
                            )
                        nc.scalar.copy(s_sb[:, h * 2048 : (h + 1) * 2048], ps[:])

                    cand = smallp.tile([128, 8 * N_CHUNKS], f32, tag="cand")
                    for c in range(N_CHUNKS):
                        nc.vector.max(
                            cand[:, c * 8 : (c + 1) * 8],
                            s_sb[:, c * CHUNK : (c + 1) * CHUNK],
                        )
                    f8a = smallp.tile([128, 8], f32, tag="f8a")
                    nc.vector.max(f8a[:], cand[:])
                    cand_mr = smallp.tile([128, 8 * N_CHUNKS], f32, tag="cmr")
                    nc.vector.match_replace(
                        out=cand_mr[:],
                        in_to_replace=f8a[:],
                        in_values=cand[:],
                        imm_value=NEG_INF,
                    )
                    f8b = smallp.tile([128, 8], f32, tag="f8b")
                    nc.vector.max(f8b[:], cand_mr[:])

                    idx16 = smallp.tile([128, K], u32, tag="idx")
                    nc.vector.max_index(idx16[:, 0:8], f8a[:], s_sb[:])
                    nc.vector.max_index(idx16[:, 8:16], f8b[:], s_sb[:])
                    nc.sync.dma_start(
                        idx_out[t * 128 : (t + 1) * 128, :], idx16[:]
                    )

    nc.compile()
    return nc


def get_program():
    global _compiled
    if _compiled is None:
        _compiled = build_program()
    return _compiled


def make_in_maps(x):
    """x: [N_SETS, M, D] float32 -> list of 8 per-core input dicts."""
    x = np.asarray(x, dtype=np.float32)
    in_maps = []
    for c in range(N_CORES):
        s, half = divmod(c, 2)
        xs = x[s]
        if half:
            xs = np.concatenate([xs[ROWS_PER_CORE:], xs[:ROWS_PER_CORE]], axis=0)
        xs = np.ascontiguousarray(xs)
        x2 = np.einsum("md,md->m", xs, xs, dtype=np.float32).astype(np.float32)
        b = (-0.5 * x2).astype(np.float32)
        b3 = np.zeros((3, M), dtype=ml_dtypes.bfloat16)
        rem = b
        for i in range(3):
            b3[i] = rem.astype(ml_dtypes.bfloat16)
            rem = rem - b3[i].astype(np.float32)
        xt = np.ascontiguousarray(xs.T)
        in_maps.append({"xt": xt, "negx2b3": b3})
    return in_maps


def assemble(per_core_idx):
    """per_core_idx: list of 8 [2048,16] u32 arrays -> (src, dst) int32."""
    src = np.empty((N_SETS, M, K), dtype=np.int64)
    for c in range(N_CORES):
        s, half = divmod(c, 2)
        idx = per_core_idx[c].astype(np.int64)
        local = (idx + half * ROWS_PER_CORE) % M
        src[s, half * ROWS_PER_CORE : (half + 1) * ROWS_PER_CORE, :] = local + s * M
    src = src.reshape(-1).astype(np.int32)
    dst = np.repeat(np.arange(N_SETS * M, dtype=np.int32), K)
    return src, dst


def run_spmd(x, trace=False, **kwargs):
    from concourse import bass_utils

    nc = get_program()
    in_maps = make_in_maps(x)
    res = bass_utils.run_bass_kernel_spmd(
        nc, in_maps, core_ids=list(range(N_CORES)), trace=trace, **kwargs
    )
    per_core = [res.results[c]["idx_out"] for c in range(N_CORES)]
    return assemble(per_core), res


def kernel(x, k):
    k = int(np.asarray(k))
    assert k == K, f"kernel hardcoded for k={K}, got {k}"
    x = np.asarray(x, dtype=np.float32)
    assert x.shape == (N_SETS, M, D), f"unexpected shape {x.shape}"
    (src, dst), _ = run_spmd(x)
    return src, dst
